# revision 15
# baseline (speedup 1.0000x reference)
"""Trainium2 Bass kernel for nn_InvNet (topk_masking, memory-bound).

Contract: kernel(**inputs) takes FULL numpy inputs (as in setup_inputs()) and
returns the FULL output tuple (loss, q, q2, q3, q4, new_targets).

Sharding: class dimension C=32768 is split across 8 NeuronCores (4096 each).
Each core:
  - streams its class-shard of all four queues through SBUF (bulk copy out),
  - computes l_neg = f_norm @ queue_shard (fp32 PE matmul) reusing the same
    queue tiles,
  - reduces per-shard top-8 (values + indices), per-shard sum(exp(x - m)),
  - row-normalizes feature/prob matrices on device and emits the
    replacement-column blocks (normalized rows gathered by a one-hot matmul)
    used for the queue scatter update.
Host merges the tiny per-shard stats (8x8 top-k candidates, logsumexp terms),
computes the scalar loss, scatters the <=256 updated queue columns and the
~1.8k sparse one-hot weights into the outputs.
"""

import sys

sys.path.insert(0, "/opt/trn_rl_repo")

import numpy as np

import concourse.bass as bass
import concourse.tile as tile
from concourse import bacc, mybir
from concourse.masks import make_identity

B = 256
F = 2048
C = 32768
P = 1041
BETA = 0.05
KNN = 6
NCORES = 8
CS = C // NCORES  # 4096 classes per core
NSLOT = 64  # max updated columns handled on-device per core
F32 = mybir.dt.float32

_PROGRAM = None
TRACE = False  # set True (e.g. from test.py) to capture an NTFF profile
LAST_RESULT = None  # BassKernelResults of the most recent run


def _normalize_rows(nc, pool, scratch_pool, x_dram, rows, d):
    """Load [rows, d] from DRAM, return list of row-normalized SBUF tiles."""
    tiles = []
    ntiles = (rows + 127) // 128
    for m in range(ntiles):
        r0 = m * 128
        r1 = min(r0 + 128, rows)
        pr = r1 - r0
        raw = pool.tile([128, d], F32, tag="rawload", name="rawload")
        nc.sync.dma_start(out=raw[:pr, :], in_=x_dram[r0:r1, :])
        sq = scratch_pool.tile([128, d], F32, tag="normscratch", name="normscratch")
        ss = pool.tile([128, 1], F32, tag="norm_ss", name="norm_ss")
        nc.scalar.activation(
            out=sq[:pr, :],
            in_=raw[:pr, :],
            func=mybir.ActivationFunctionType.Square,
            accum_out=ss[:pr, :],
        )
        nrm = pool.tile([128, 1], F32, tag="norm_n", name="norm_n")
        nc.scalar.activation(
            out=nrm[:pr, :], in_=ss[:pr, :], func=mybir.ActivationFunctionType.Sqrt
        )
        rinv = pool.tile([128, 1], F32, tag="norm_rinv", name="norm_rinv")
        nc.vector.reciprocal(rinv[:pr, :], nrm[:pr, :])
        nt = pool.tile([128, d], F32, tag=f"normed_{x_dram.name}_{m}", name=f"normed_{x_dram.name}_{m}")
        nc.vector.tensor_scalar_mul(nt[:pr, :], raw[:pr, :], rinv[:pr, :])
        tiles.append(nt)
    return tiles


def _build_program():
    nc = bacc.Bacc("TRN2", target_bir_lowering=False, debug=False, num_devices=NCORES)

    q_in = nc.dram_tensor("q", [F, CS], F32, kind="ExternalInput")
    q2_in = nc.dram_tensor("q2", [F, CS], F32, kind="ExternalInput")
    q3_in = nc.dram_tensor("q3", [P, CS], F32, kind="ExternalInput")
    q4_in = nc.dram_tensor("q4", [P, CS], F32, kind="ExternalInput")
    f_in = nc.dram_tensor("feat", [B, F], F32, kind="ExternalInput")
    ema_in = nc.dram_tensor("ema", [B, F], F32, kind="ExternalInput")
    p_in = nc.dram_tensor("prob", [B, P], F32, kind="ExternalInput")
    pe_in = nc.dram_tensor("probe", [B, P], F32, kind="ExternalInput")
    oh_in = nc.dram_tensor("onehot", [B, NSLOT], F32, kind="ExternalInput")

    q_out = nc.dram_tensor("q_out", [F, CS], F32, kind="ExternalOutput")
    q2_out = nc.dram_tensor("q2_out", [F, CS], F32, kind="ExternalOutput")
    q3_out = nc.dram_tensor("q3_out", [P, CS], F32, kind="ExternalOutput")
    q4_out = nc.dram_tensor("q4_out", [P, CS], F32, kind="ExternalOutput")
    repl1 = nc.dram_tensor("repl1", [F, NSLOT], F32, kind="ExternalOutput")
    repl2 = nc.dram_tensor("repl2", [F, NSLOT], F32, kind="ExternalOutput")
    repl3 = nc.dram_tensor("repl3", [P, NSLOT], F32, kind="ExternalOutput")
    repl4 = nc.dram_tensor("repl4", [P, NSLOT], F32, kind="ExternalOutput")
    cand_vals = nc.dram_tensor("cand_vals", [B, 8], F32, kind="ExternalOutput")
    cand_idx = nc.dram_tensor("cand_idx", [B, 8], mybir.dt.uint32, kind="ExternalOutput")
    sumexp = nc.dram_tensor("sumexp", [B, 1], F32, kind="ExternalOutput")
    lpos = nc.dram_tensor("lpos", [B, 1], F32, kind="ExternalOutput")

    HALF = CS // 2  # 2048 col half-tiles for streaming

    with tile.TileContext(nc) as tc:
        from contextlib import ExitStack
        with ExitStack() as ctx:
            pp = ctx.enter_context(tc.tile_pool(name="persist", bufs=1))
            npool = ctx.enter_context(tc.tile_pool(name="normed", bufs=1))
            smalls = ctx.enter_context(tc.tile_pool(name="smalls", bufs=8))
            qp = ctx.enter_context(tc.tile_pool(name="qpool", bufs=8))
            cp = ctx.enter_context(tc.tile_pool(name="cpool", bufs=5))
            mps = ctx.enter_context(tc.tile_pool(name="mmpsum", bufs=8, space="PSUM"))

            acc = [pp.tile([128, CS], F32, tag=f"acc{m}", name=f"acc{m}")
                   for m in range(2)]
            fT = [pp.tile([128, B], F32, tag=f"fT{k}", name=f"fT{k}")
                  for k in range(F // 128)]
            ident = pp.tile([128, 128], F32, tag="ident", name="ident")
            make_identity(nc, ident[:])
            cv = pp.tile([128, 16], F32, tag="cv", name="cv")
            ci = pp.tile([128, 16], mybir.dt.uint32, tag="ci", name="ci")
            sv = pp.tile([128, 8], F32, tag="sv", name="sv")
            ohsb = pp.tile([128, 2 * NSLOT], F32, tag="ohsb", name="ohsb")

            def normalize(x_dram, d, m):
                """Load row-tile m of x and return a normalized SBUF tile."""
                r0, r1 = m * 128, min((m + 1) * 128, B)
                pr = r1 - r0
                raw = cp.tile([128, HALF], F32, tag="ctile", name="rawload")
                nc.sync.dma_start(out=raw[:pr, :d], in_=x_dram[r0:r1, :])
                sq = cp.tile([128, HALF], F32, tag="ctile", name="sqscratch")
                nv = smalls.tile([128, 4], F32, tag="nrm", name="nrm")
                nc.scalar.activation(
                    out=sq[:pr, :d], in_=raw[:pr, :d],
                    func=mybir.ActivationFunctionType.Square,
                    accum_out=nv[:pr, 0:1],
                )
                nc.scalar.activation(
                    out=nv[:pr, 1:2], in_=nv[:pr, 0:1],
                    func=mybir.ActivationFunctionType.Sqrt,
                )
                nc.vector.reciprocal(nv[:pr, 2:3], nv[:pr, 1:2])
                nt = npool.tile(
                    [128, d], F32, tag=f"normed_{x_dram.name}_{m}",
                    name=f"normed_{x_dram.name}_{m}",
                )
                nc.vector.tensor_scalar_mul(nt[:pr, :], raw[:pr, :d], nv[:pr, 2:3])
                return nt

            # --- critical path first: f -> fN -> fT (feeds all matmuls) ---
            fN = [normalize(f_in, F, m) for m in range(2)]
            for k in range(F // 128):
                for m in range(2):
                    pt = mps.tile([128, 512], F32, tag="mmps", name="tpsum")
                    nc.tensor.transpose(
                        out=pt[:, :128],
                        in_=fN[m][:, k * 128:(k + 1) * 128],
                        identity=ident[:],
                    )
                    nc.vector.tensor_copy(
                        out=fT[k][:, m * 128:(m + 1) * 128], in_=pt[:, :128]
                    )
            nc.sync.dma_start(out=ohsb[:, :NSLOT], in_=oh_in[0:128, :])
            nc.sync.dma_start(out=ohsb[:, NSLOT:], in_=oh_in[128:256, :])

            emaN = pN = peN = None

            def q_chunk(c):
                """Stream 2 row-tiles of `queue`: DMA in/out + matmul chunk.

                PSUM only fits 8 [128,512] banks, so accumulate k-pairs per
                m-tile sequentially: m0 matmuls, evict to acc, then m1.
                """
                tiles = []
                for kk in range(2):
                    k = 2 * c + kk
                    for h in range(2):
                        qt = qp.tile([128, HALF], F32, tag="qtile", name="qtile")
                        nc.sync.dma_start(
                            out=qt[:],
                            in_=q_in[k * 128:(k + 1) * 128,
                                     h * HALF:(h + 1) * HALF],
                        )
                        nc.gpsimd.dma_start(
                            out=q_out[k * 128:(k + 1) * 128,
                                      h * HALF:(h + 1) * HALF],
                            in_=qt[:],
                        )
                        tiles.append(qt)
                for m in range(2):
                    psums = [
                        mps.tile([128, 512], F32, tag="mmps", name="mmps")
                        for _ in range(8)
                    ]
                    for kk in range(2):
                        k = 2 * c + kk
                        for nb in range(8):
                            nc.tensor.matmul(
                                out=psums[nb][:],
                                lhsT=fT[k][:, m * 128:(m + 1) * 128],
                                rhs=tiles[2 * kk + nb // 4][
                                    :, (nb % 4) * 512:(nb % 4 + 1) * 512
                                ],
                                start=(kk == 0),
                                stop=(kk == 1),
                            )
                    for nb in range(8):
                        dstap = acc[m][:, nb * 512:(nb + 1) * 512]
                        if c == 0:
                            nc.vector.tensor_copy(out=dstap, in_=psums[nb][:])
                        else:
                            nc.vector.tensor_tensor(
                                out=dstap, in0=dstap, in1=psums[nb][:],
                                op=mybir.AluOpType.add,
                            )

            def copy_unit(src, dst, t, h, rows):
                r0 = t * 128
                pr = min(128, rows - r0)
                nc.sync.dma_start(
                    out=dst[r0:r0 + pr, h * HALF:(h + 1) * HALF],
                    in_=src[r0:r0 + pr, h * HALF:(h + 1) * HALF],
                )

            def repl_unit(src_tiles, dst, d):
                nt = (d + 127) // 128
                for t in range(nt):
                    f0 = t * 128
                    mt = min(128, d - f0)
                    ps = mps.tile([128, 512], F32, tag="mmps", name="replpsum")
                    for m in range(2):
                        nc.tensor.matmul(
                            out=ps[:mt, :NSLOT],
                            lhsT=src_tiles[m][:, f0:f0 + mt],
                            rhs=ohsb[:, m * NSLOT:(m + 1) * NSLOT],
                            start=(m == 0),
                            stop=(m == 1),
                        )
                    rs = smalls.tile([128, NSLOT], F32, tag="replsb", name="replsb")
                    nc.vector.tensor_copy(out=rs[:mt, :], in_=ps[:mt, :NSLOT])
                    nc.gpsimd.dma_start(out=dst[f0:f0 + mt, :], in_=rs[:mt, :])

            # --- interleaved emission: q-chunks + most d2d copies mixed,
            # holding back a tail reserve of copies to cover the stats ---
            import itertools
            q2u = [(q2_in, q2_out, t, h, F) for t in range(16) for h in range(2)]
            q3u = [(q3_in, q3_out, t, h, P) for t in range(9) for h in range(2)]
            q4u = [(q4_in, q4_out, t, h, P) for t in range(9) for h in range(2)]
            mixed = [u for tri in itertools.zip_longest(q2u, q3u, q4u)
                     for u in tri if u is not None]
            reserve = mixed[-12:]
            mixed = mixed[:-12]
            per_step = (len(mixed) + 7) // 8
            for i in range(8):
                q_chunk(i)
                for u in mixed[i * per_step:(i + 1) * per_step]:
                    copy_unit(*u)
                if i == 1:
                    emaN = [normalize(ema_in, F, m) for m in range(2)]
                if i == 2:
                    pN = [normalize(p_in, P, m) for m in range(2)]
                    peN = [normalize(pe_in, P, m) for m in range(2)]
                if i == 3:
                    # l_pos = rowsum(fN * emaN)
                    for m in range(2):
                        prod = cp.tile([128, HALF], F32, tag="ctile", name="lposprod")
                        nc.vector.tensor_tensor(
                            out=prod[:], in0=fN[m][:], in1=emaN[m][:],
                            op=mybir.AluOpType.mult,
                        )
                        lp = smalls.tile([128, 4], F32, tag="nrm", name="lpossb")
                        nc.vector.tensor_reduce(
                            out=lp[:, 0:1], in_=prod[:], axis=mybir.AxisListType.X,
                            op=mybir.AluOpType.add,
                        )
                        nc.gpsimd.dma_start(
                            out=lpos[m * 128:(m + 1) * 128, :], in_=lp[:, 0:1]
                        )
                if i == 4:
                    repl_unit(emaN, repl1, F)
                    repl_unit(fN, repl2, F)
                if i == 5:
                    repl_unit(pN, repl3, P)
                    repl_unit(peN, repl4, P)

            # --- per-shard stats from acc ---
            for m in range(2):
                cvm = cv[:, m * 8:(m + 1) * 8]
                nc.vector.max(out=cvm, in_=acc[m][:])
                cim = ci[:, m * 8:(m + 1) * 8]
                nc.vector.max_index(out=cim, in_max=cvm, in_values=acc[m][:])
                bias = sv[:, m:m + 1]
                nc.vector.tensor_scalar_mul(bias, cv[:, m * 8:m * 8 + 1], -1.0 / BETA)
                se0 = sv[:, 2 + 2 * m:3 + 2 * m]
                se1 = sv[:, 3 + 2 * m:4 + 2 * m]
                for h in range(2):
                    esc = cp.tile([128, HALF], F32, tag="ctile", name="expscratch")
                    nc.scalar.activation(
                        out=esc[:],
                        in_=acc[m][:, h * HALF:(h + 1) * HALF],
                        func=mybir.ActivationFunctionType.Exp,
                        bias=bias,
                        scale=1.0 / BETA,
                        accum_out=(se0 if h == 0 else se1),
                    )
                sem = sv[:, 6 + m:7 + m]
                nc.vector.tensor_tensor(
                    out=sem, in0=se0, in1=se1, op=mybir.AluOpType.add
                )
                sl = slice(m * 128, (m + 1) * 128)
                nc.gpsimd.dma_start(out=cand_vals[sl, :], in_=cvm)
                nc.gpsimd.dma_start(out=cand_idx[sl, :], in_=cim)
                nc.gpsimd.dma_start(out=sumexp[sl, :], in_=sem)

            # tail reserve: keeps DMA busy while stats run
            for u in reserve:
                copy_unit(*u)

    nc.compile()
    return nc


def _get_program():
    global _PROGRAM
    if _PROGRAM is None:
        _PROGRAM = _build_program()
    return _PROGRAM


def _softmax_rows(x):
    m = x.max(axis=1, keepdims=True)
    e = np.exp(x - m)
    return e / e.sum(axis=1, keepdims=True)


def _install_ntff_hook():
    """This image's antenv lacks axon_hooks; shim it so trace=True works."""
    import types

    try:
        from antenv.axon_hooks import get_axon_ntff_profile_hook  # noqa: F401
        return
    except ImportError:
        pass
    if "/root/.axon_site" not in sys.path:
        sys.path.insert(0, "/root/.axon_site")
    try:
        from trn_agent_boot.trn_boot import _ntff_profile_via_ctypes

        hook = _ntff_profile_via_ctypes("/opt/axon/libaxon_pjrt.so")
    except Exception:
        hook = None
    import antenv

    mod = types.ModuleType("antenv.axon_hooks")
    mod.get_axon_ntff_profile_hook = lambda: hook
    mod.set_axon_ntff_profile_hook = lambda h: None
    sys.modules["antenv.axon_hooks"] = mod
    antenv.axon_hooks = mod


def kernel(**inputs):
    from concourse.bass_utils import run_bass_kernel_spmd

    if TRACE:
        _install_ntff_hook()

    f = np.ascontiguousarray(np.asarray(inputs["inputs_feature"], dtype=np.float32))
    ema = np.ascontiguousarray(np.asarray(inputs["inputs_ema"], dtype=np.float32))
    prob = np.ascontiguousarray(np.asarray(inputs["prob"], dtype=np.float32))
    prob_ema = np.ascontiguousarray(np.asarray(inputs["prob_ema"], dtype=np.float32))
    queue = np.asarray(inputs["queue"], dtype=np.float32)
    queue_2 = np.asarray(inputs["queue_2"], dtype=np.float32)
    queue_3 = np.asarray(inputs["queue_3"], dtype=np.float32)
    queue_4 = np.asarray(inputs["queue_4"], dtype=np.float32)
    targets_in = np.asarray(inputs["targets"])
    targets = targets_in.astype(np.int64)
    epoch = int(np.asarray(inputs["epoch"]))

    nc = _get_program()

    # host-side slot assignment: which rows' target columns live on which core
    owner = targets // CS
    slot_rows = [np.nonzero(owner == s)[0] for s in range(NCORES)]
    use_device_repl = all(len(r) <= NSLOT for r in slot_rows)

    in_maps = []
    for s in range(NCORES):
        sl = slice(s * CS, (s + 1) * CS)
        onehot = np.zeros((B, NSLOT), np.float32)
        if use_device_repl:
            rows = slot_rows[s]
            onehot[rows, np.arange(len(rows))] = 1.0
        in_maps.append({
            "q": np.ascontiguousarray(queue[:, sl]),
            "q2": np.ascontiguousarray(queue_2[:, sl]),
            "q3": np.ascontiguousarray(queue_3[:, sl]),
            "q4": np.ascontiguousarray(queue_4[:, sl]),
            "feat": f,
            "ema": ema,
            "prob": prob,
            "probe": prob_ema,
            "onehot": onehot,
        })

    res = run_bass_kernel_spmd(
        nc, in_maps, core_ids=list(range(NCORES)), trace=TRACE
    )
    global LAST_RESULT
    LAST_RESULT = res
    outs = res.results

    # ---- assemble bulk outputs ----
    q = np.empty((F, C), np.float32)
    q2 = np.empty((F, C), np.float32)
    q3 = np.empty((P, C), np.float32)
    q4 = np.empty((P, C), np.float32)
    for s in range(NCORES):
        sl = slice(s * CS, (s + 1) * CS)
        q[:, sl] = outs[s]["q_out"]
        q2[:, sl] = outs[s]["q2_out"]
        q3[:, sl] = outs[s]["q3_out"]
        q4[:, sl] = outs[s]["q4_out"]

    # ---- column updates (scatter the <=256 replaced columns) ----
    if use_device_repl:
        for s in range(NCORES):
            rows = slot_rows[s]
            if len(rows) == 0:
                continue
            cols = targets[rows]
            n = len(rows)
            q[:, cols] = outs[s]["repl1"][:, :n]
            q2[:, cols] = outs[s]["repl2"][:, :n]
            q3[:, cols] = outs[s]["repl3"][:, :n]
            q4[:, cols] = outs[s]["repl4"][:, :n]
    else:  # fallback: host-normalized replacements (pathological target skew)
        def hnorm(x):
            return x / np.sqrt((x * x).sum(axis=1, keepdims=True))
        q[:, targets] = hnorm(ema).T
        q2[:, targets] = hnorm(f).T
        q3[:, targets] = hnorm(prob).T
        q4[:, targets] = hnorm(prob_ema).T

    # ---- merge per-shard stats ----
    l_pos = outs[0]["lpos"][:, 0]  # [B]
    shard_vals = np.stack([outs[s]["cand_vals"] for s in range(NCORES)])  # [S,B,8]
    shard_idx = np.stack(
        [outs[s]["cand_idx"].astype(np.int64) for s in range(NCORES)]
    )  # [S,B,8] local col idx
    shard_se = np.stack([outs[s]["sumexp"][:, 0] for s in range(NCORES)])  # [S,B]
    m_s = shard_vals[:, :, 0]  # [S,B] per-shard max of l_neg

    # candidates in concat space (col 0 = l_pos, cols 1.. = l_neg).
    # l_pos candidate goes FIRST so argsort tie-break matches jax (lowest
    # index first on ties).
    cvals = np.concatenate(
        [l_pos[:, None]]
        + [shard_vals[s] for s in range(NCORES)], axis=1
    )  # [B, 65]
    cidx = np.concatenate(
        [np.zeros((B, 1), np.int64)]
        + [shard_idx[s] + s * CS + 1 for s in range(NCORES)], axis=1
    )  # [B, 65]
    order = np.argsort(-cvals, axis=1, kind="stable")[:, :KNN + 1]
    rows_ar = np.arange(B)[:, None]
    topv = cvals[rows_ar, order]  # [B, 7] descending
    topi = cidx[rows_ar, order]  # [B, 7]

    # global logsumexp of x = logits/BETA
    M = m_s.max(axis=0)  # [B]
    S_total = (shard_se * np.exp((m_s - M[None, :]) / BETA)).sum(axis=0)
    S_total = S_total + np.exp((l_pos - M) / BETA)
    lse = M / BETA + np.log(S_total)  # [B]

    # x[r, t_r]: t==0 -> l_pos else f_norm . queue[:, t-1]  (pre-update queue)
    fN_host = f / np.sqrt((f * f).sum(axis=1, keepdims=True))
    tcols = np.clip(targets - 1, 0, C - 1)
    qcols = queue[:, tcols]  # [F, B]
    tval = (fN_host * qcols.T).sum(axis=1)
    tval = np.where(targets == 0, l_pos, tval)
    xt = tval / BETA

    if KNN > 0 and epoch >= 20:
        w = _softmax_rows(topv[:, 1:] / BETA)  # [B, 6]
        neq = (topi[:, 1:] != targets[:, None])
        loss_rows = (lse - xt) + (w * (lse[:, None] - topv[:, 1:] / BETA) * neq).sum(
            axis=1
        )
        loss = np.float32(loss_rows.mean())
        oh = np.zeros((B, C + 1), np.float32)
        oh[rows_ar, topi[:, 1:]] = w.astype(np.float32)
        oh[np.arange(B), targets] = 1.0
        new_targets = oh
    else:
        loss = np.float32((lse - xt).mean())
        new_targets = targets_in.copy()

    return (loss, q, q2, q3, q4, new_targets)


# revision 16
# speedup vs baseline: 1.5597x; 1.5597x over previous
"""Trainium2 Bass kernel for nn_InvNet (topk_masking, memory-bound).

Contract: kernel(**inputs) takes FULL numpy inputs (as in setup_inputs()) and
returns the FULL output tuple (loss, q, q2, q3, q4, new_targets).

Sharding: class dimension C=32768 is split across 8 NeuronCores (4096 each).
Each core:
  - streams its class-shard of all four queues through SBUF (bulk copy out),
  - computes l_neg = f_norm @ queue_shard (fp32 PE matmul) reusing the same
    queue tiles,
  - reduces per-shard top-8 (values + indices), per-shard sum(exp(x - m)),
  - row-normalizes feature/prob matrices on device and emits the
    replacement-column blocks (normalized rows gathered by a one-hot matmul)
    used for the queue scatter update.
Host merges the tiny per-shard stats (8x8 top-k candidates, logsumexp terms),
computes the scalar loss, scatters the <=256 updated queue columns and the
~1.8k sparse one-hot weights into the outputs.
"""

import sys

sys.path.insert(0, "/opt/trn_rl_repo")

import numpy as np

import concourse.bass as bass
import concourse.tile as tile
from concourse import bacc, mybir
from concourse.masks import make_identity

B = 256
F = 2048
C = 32768
P = 1041
BETA = 0.05
KNN = 6
NCORES = 8
CS = C // NCORES  # 4096 classes per core
NSLOT = 64  # max updated columns handled on-device per core
F32 = mybir.dt.float32

_PROGRAM = None
TRACE = False  # set True (e.g. from test.py) to capture an NTFF profile
LAST_RESULT = None  # BassKernelResults of the most recent run


def _normalize_rows(nc, pool, scratch_pool, x_dram, rows, d):
    """Load [rows, d] from DRAM, return list of row-normalized SBUF tiles."""
    tiles = []
    ntiles = (rows + 127) // 128
    for m in range(ntiles):
        r0 = m * 128
        r1 = min(r0 + 128, rows)
        pr = r1 - r0
        raw = pool.tile([128, d], F32, tag="rawload", name="rawload")
        nc.sync.dma_start(out=raw[:pr, :], in_=x_dram[r0:r1, :])
        sq = scratch_pool.tile([128, d], F32, tag="normscratch", name="normscratch")
        ss = pool.tile([128, 1], F32, tag="norm_ss", name="norm_ss")
        nc.scalar.activation(
            out=sq[:pr, :],
            in_=raw[:pr, :],
            func=mybir.ActivationFunctionType.Square,
            accum_out=ss[:pr, :],
        )
        nrm = pool.tile([128, 1], F32, tag="norm_n", name="norm_n")
        nc.scalar.activation(
            out=nrm[:pr, :], in_=ss[:pr, :], func=mybir.ActivationFunctionType.Sqrt
        )
        rinv = pool.tile([128, 1], F32, tag="norm_rinv", name="norm_rinv")
        nc.vector.reciprocal(rinv[:pr, :], nrm[:pr, :])
        nt = pool.tile([128, d], F32, tag=f"normed_{x_dram.name}_{m}", name=f"normed_{x_dram.name}_{m}")
        nc.vector.tensor_scalar_mul(nt[:pr, :], raw[:pr, :], rinv[:pr, :])
        tiles.append(nt)
    return tiles


def _build_program():
    nc = bacc.Bacc("TRN2", target_bir_lowering=False, debug=False, num_devices=NCORES)

    q_in = nc.dram_tensor("q", [F, CS], F32, kind="ExternalInput")
    q2_in = nc.dram_tensor("q2", [F, CS], F32, kind="ExternalInput")
    q3_in = nc.dram_tensor("q3", [P, CS], F32, kind="ExternalInput")
    q4_in = nc.dram_tensor("q4", [P, CS], F32, kind="ExternalInput")
    f_in = nc.dram_tensor("feat", [B, F], F32, kind="ExternalInput")
    ema_in = nc.dram_tensor("ema", [B, F], F32, kind="ExternalInput")
    p_in = nc.dram_tensor("prob", [B, P], F32, kind="ExternalInput")
    pe_in = nc.dram_tensor("probe", [B, P], F32, kind="ExternalInput")
    oh_in = nc.dram_tensor("onehot", [B, NSLOT], F32, kind="ExternalInput")

    q_out = nc.dram_tensor("q_out", [F, CS], F32, kind="ExternalOutput")
    q2_out = nc.dram_tensor("q2_out", [F, CS], F32, kind="ExternalOutput")
    q3_out = nc.dram_tensor("q3_out", [P, CS], F32, kind="ExternalOutput")
    q4_out = nc.dram_tensor("q4_out", [P, CS], F32, kind="ExternalOutput")
    repl1 = nc.dram_tensor("repl1", [F, NSLOT], F32, kind="ExternalOutput")
    repl2 = nc.dram_tensor("repl2", [F, NSLOT], F32, kind="ExternalOutput")
    repl3 = nc.dram_tensor("repl3", [P, NSLOT], F32, kind="ExternalOutput")
    repl4 = nc.dram_tensor("repl4", [P, NSLOT], F32, kind="ExternalOutput")
    cand_vals = nc.dram_tensor("cand_vals", [B, 8], F32, kind="ExternalOutput")
    cand_idx = nc.dram_tensor("cand_idx", [B, 8], mybir.dt.uint32, kind="ExternalOutput")
    sumexp = nc.dram_tensor("sumexp", [B, 1], F32, kind="ExternalOutput")
    lpos = nc.dram_tensor("lpos", [B, 1], F32, kind="ExternalOutput")

    HALF = CS // 2  # 2048 col half-tiles for streaming

    with tile.TileContext(nc) as tc:
        from contextlib import ExitStack
        with ExitStack() as ctx:
            pp = ctx.enter_context(tc.tile_pool(name="persist", bufs=1))
            npool = ctx.enter_context(tc.tile_pool(name="normed", bufs=1))
            smalls = ctx.enter_context(tc.tile_pool(name="smalls", bufs=8))
            qp = ctx.enter_context(tc.tile_pool(name="qpool", bufs=10))
            cp = ctx.enter_context(tc.tile_pool(name="cpool", bufs=3))
            mps = ctx.enter_context(tc.tile_pool(name="mmpsum", bufs=8, space="PSUM"))

            acc = [pp.tile([128, CS], F32, tag=f"acc{m}", name=f"acc{m}")
                   for m in range(2)]
            fT = [pp.tile([128, B], F32, tag=f"fT{k}", name=f"fT{k}")
                  for k in range(F // 128)]
            ident = pp.tile([128, 128], F32, tag="ident", name="ident")
            make_identity(nc, ident[:])
            cv = pp.tile([128, 16], F32, tag="cv", name="cv")
            ci = pp.tile([128, 16], mybir.dt.uint32, tag="ci", name="ci")
            sv = pp.tile([128, 8], F32, tag="sv", name="sv")
            ohsb = pp.tile([128, 2 * NSLOT], F32, tag="ohsb", name="ohsb")

            def normalize(x_dram, d, m):
                """Load row-tile m of x and return a normalized SBUF tile."""
                r0, r1 = m * 128, min((m + 1) * 128, B)
                pr = r1 - r0
                raw = cp.tile([128, HALF], F32, tag="ctile", name="rawload")
                nc.sync.dma_start(out=raw[:pr, :d], in_=x_dram[r0:r1, :])
                sq = cp.tile([128, HALF], F32, tag="ctile", name="sqscratch")
                nv = smalls.tile([128, 4], F32, tag="nrm", name="nrm")
                nc.scalar.activation(
                    out=sq[:pr, :d], in_=raw[:pr, :d],
                    func=mybir.ActivationFunctionType.Square,
                    accum_out=nv[:pr, 0:1],
                )
                nc.scalar.activation(
                    out=nv[:pr, 1:2], in_=nv[:pr, 0:1],
                    func=mybir.ActivationFunctionType.Sqrt,
                )
                nc.vector.reciprocal(nv[:pr, 2:3], nv[:pr, 1:2])
                nt = npool.tile(
                    [128, d], F32, tag=f"normed_{x_dram.name}_{m}",
                    name=f"normed_{x_dram.name}_{m}",
                )
                nc.vector.tensor_scalar_mul(nt[:pr, :], raw[:pr, :d], nv[:pr, 2:3])
                return nt

            # --- critical path first: f -> fN -> fT (feeds all matmuls) ---
            fN = [normalize(f_in, F, m) for m in range(2)]
            for k in range(F // 128):
                for m in range(2):
                    pt = mps.tile([128, 512], F32, tag="mmps", name="tpsum")
                    nc.tensor.transpose(
                        out=pt[:, :128],
                        in_=fN[m][:, k * 128:(k + 1) * 128],
                        identity=ident[:],
                    )
                    nc.vector.tensor_copy(
                        out=fT[k][:, m * 128:(m + 1) * 128], in_=pt[:, :128]
                    )
            nc.sync.dma_start(out=ohsb[:, :NSLOT], in_=oh_in[0:128, :])
            nc.sync.dma_start(out=ohsb[:, NSLOT:], in_=oh_in[128:256, :])

            emaN = pN = peN = None

            def q_chunk(c):
                """Stream 2 row-tiles of `queue`: DMA in/out + matmul chunk.

                PSUM only fits 8 [128,512] banks, so accumulate k-pairs per
                m-tile sequentially: m0 matmuls, evict to acc, then m1.
                """
                tiles = []
                for kk in range(2):
                    k = 2 * c + kk
                    for h in range(2):
                        qt = qp.tile([128, HALF], F32, tag="qtile", name="qtile")
                        nc.sync.dma_start(
                            out=qt[:],
                            in_=q_in[k * 128:(k + 1) * 128,
                                     h * HALF:(h + 1) * HALF],
                        )
                        nc.gpsimd.dma_start(
                            out=q_out[k * 128:(k + 1) * 128,
                                      h * HALF:(h + 1) * HALF],
                            in_=qt[:],
                        )
                        tiles.append(qt)
                for m in range(2):
                    psums = [
                        mps.tile([128, 512], F32, tag="mmps", name="mmps")
                        for _ in range(8)
                    ]
                    for kk in range(2):
                        k = 2 * c + kk
                        for nb in range(8):
                            nc.tensor.matmul(
                                out=psums[nb][:],
                                lhsT=fT[k][:, m * 128:(m + 1) * 128],
                                rhs=tiles[2 * kk + nb // 4][
                                    :, (nb % 4) * 512:(nb % 4 + 1) * 512
                                ],
                                start=(kk == 0),
                                stop=(kk == 1),
                            )
                    for nb in range(8):
                        dstap = acc[m][:, nb * 512:(nb + 1) * 512]
                        if c == 0:
                            nc.vector.tensor_copy(out=dstap, in_=psums[nb][:])
                        else:
                            nc.vector.tensor_tensor(
                                out=dstap, in0=dstap, in1=psums[nb][:],
                                op=mybir.AluOpType.add,
                            )

            def copy_unit(src, dst, t, h, rows):
                r0 = t * 128
                pr = min(128, rows - r0)
                nc.sync.dma_start(
                    out=dst[r0:r0 + pr, h * HALF:(h + 1) * HALF],
                    in_=src[r0:r0 + pr, h * HALF:(h + 1) * HALF],
                )

            def repl_unit(src_tiles, dst, d):
                nt = (d + 127) // 128
                for t in range(nt):
                    f0 = t * 128
                    mt = min(128, d - f0)
                    ps = mps.tile([128, 512], F32, tag="mmps", name="replpsum")
                    for m in range(2):
                        nc.tensor.matmul(
                            out=ps[:mt, :NSLOT],
                            lhsT=src_tiles[m][:, f0:f0 + mt],
                            rhs=ohsb[:, m * NSLOT:(m + 1) * NSLOT],
                            start=(m == 0),
                            stop=(m == 1),
                        )
                    rs = smalls.tile([128, NSLOT], F32, tag="replsb", name="replsb")
                    nc.vector.tensor_copy(out=rs[:mt, :], in_=ps[:mt, :NSLOT])
                    nc.gpsimd.dma_start(out=dst[f0:f0 + mt, :], in_=rs[:mt, :])

            # --- interleaved emission: q-chunks + most d2d copies mixed,
            # holding back a tail reserve of copies to cover the stats ---
            import itertools
            q2u = [(q2_in, q2_out, t, h, F) for t in range(16) for h in range(2)]
            q3u = [(q3_in, q3_out, t, h, P) for t in range(9) for h in range(2)]
            q4u = [(q4_in, q4_out, t, h, P) for t in range(9) for h in range(2)]
            mixed = [u for tri in itertools.zip_longest(q2u, q3u, q4u)
                     for u in tri if u is not None]
            reserve = mixed[-12:]
            mixed = mixed[:-12]
            per_step = (len(mixed) + 7) // 8
            for i in range(8):
                q_chunk(i)
                for u in mixed[i * per_step:(i + 1) * per_step]:
                    copy_unit(*u)
                if i == 1:
                    emaN = [normalize(ema_in, F, m) for m in range(2)]
                if i == 2:
                    pN = [normalize(p_in, P, m) for m in range(2)]
                    peN = [normalize(pe_in, P, m) for m in range(2)]
                if i == 3:
                    # l_pos = rowsum(fN * emaN)
                    for m in range(2):
                        prod = cp.tile([128, HALF], F32, tag="ctile", name="lposprod")
                        nc.vector.tensor_tensor(
                            out=prod[:], in0=fN[m][:], in1=emaN[m][:],
                            op=mybir.AluOpType.mult,
                        )
                        lp = smalls.tile([128, 4], F32, tag="nrm", name="lpossb")
                        nc.vector.tensor_reduce(
                            out=lp[:, 0:1], in_=prod[:], axis=mybir.AxisListType.X,
                            op=mybir.AluOpType.add,
                        )
                        nc.gpsimd.dma_start(
                            out=lpos[m * 128:(m + 1) * 128, :], in_=lp[:, 0:1]
                        )
                if i == 4:
                    repl_unit(emaN, repl1, F)
                    repl_unit(fN, repl2, F)
                if i == 5:
                    repl_unit(pN, repl3, P)
                    repl_unit(peN, repl4, P)

            # --- per-shard stats from acc ---
            for m in range(2):
                cvm = cv[:, m * 8:(m + 1) * 8]
                nc.vector.max(out=cvm, in_=acc[m][:])
                cim = ci[:, m * 8:(m + 1) * 8]
                nc.vector.max_index(out=cim, in_max=cvm, in_values=acc[m][:])
                bias = sv[:, m:m + 1]
                nc.vector.tensor_scalar_mul(bias, cv[:, m * 8:m * 8 + 1], -1.0 / BETA)
                se0 = sv[:, 2 + 2 * m:3 + 2 * m]
                se1 = sv[:, 3 + 2 * m:4 + 2 * m]
                for h in range(2):
                    esc = cp.tile([128, HALF], F32, tag="ctile", name="expscratch")
                    nc.scalar.activation(
                        out=esc[:],
                        in_=acc[m][:, h * HALF:(h + 1) * HALF],
                        func=mybir.ActivationFunctionType.Exp,
                        bias=bias,
                        scale=1.0 / BETA,
                        accum_out=(se0 if h == 0 else se1),
                    )
                sem = sv[:, 6 + m:7 + m]
                nc.vector.tensor_tensor(
                    out=sem, in0=se0, in1=se1, op=mybir.AluOpType.add
                )
                sl = slice(m * 128, (m + 1) * 128)
                nc.gpsimd.dma_start(out=cand_vals[sl, :], in_=cvm)
                nc.gpsimd.dma_start(out=cand_idx[sl, :], in_=cim)
                nc.gpsimd.dma_start(out=sumexp[sl, :], in_=sem)

            # tail reserve: keeps DMA busy while stats run
            for u in reserve:
                copy_unit(*u)

    nc.compile()
    return nc


def _get_program():
    global _PROGRAM
    if _PROGRAM is None:
        _PROGRAM = _build_program()
    return _PROGRAM


def _softmax_rows(x):
    m = x.max(axis=1, keepdims=True)
    e = np.exp(x - m)
    return e / e.sum(axis=1, keepdims=True)


def _install_ntff_hook():
    """This image's antenv lacks axon_hooks; shim it so trace=True works."""
    import types

    try:
        from antenv.axon_hooks import get_axon_ntff_profile_hook  # noqa: F401
        return
    except ImportError:
        pass
    if "/root/.axon_site" not in sys.path:
        sys.path.insert(0, "/root/.axon_site")
    try:
        from trn_agent_boot.trn_boot import _ntff_profile_via_ctypes

        hook = _ntff_profile_via_ctypes("/opt/axon/libaxon_pjrt.so")
    except Exception:
        hook = None
    import antenv

    mod = types.ModuleType("antenv.axon_hooks")
    mod.get_axon_ntff_profile_hook = lambda: hook
    mod.set_axon_ntff_profile_hook = lambda h: None
    sys.modules["antenv.axon_hooks"] = mod
    antenv.axon_hooks = mod


def kernel(**inputs):
    from concourse.bass_utils import run_bass_kernel_spmd

    if TRACE:
        _install_ntff_hook()

    f = np.ascontiguousarray(np.asarray(inputs["inputs_feature"], dtype=np.float32))
    ema = np.ascontiguousarray(np.asarray(inputs["inputs_ema"], dtype=np.float32))
    prob = np.ascontiguousarray(np.asarray(inputs["prob"], dtype=np.float32))
    prob_ema = np.ascontiguousarray(np.asarray(inputs["prob_ema"], dtype=np.float32))
    queue = np.asarray(inputs["queue"], dtype=np.float32)
    queue_2 = np.asarray(inputs["queue_2"], dtype=np.float32)
    queue_3 = np.asarray(inputs["queue_3"], dtype=np.float32)
    queue_4 = np.asarray(inputs["queue_4"], dtype=np.float32)
    targets_in = np.asarray(inputs["targets"])
    targets = targets_in.astype(np.int64)
    epoch = int(np.asarray(inputs["epoch"]))

    nc = _get_program()

    # host-side slot assignment: which rows' target columns live on which core
    owner = targets // CS
    slot_rows = [np.nonzero(owner == s)[0] for s in range(NCORES)]
    use_device_repl = all(len(r) <= NSLOT for r in slot_rows)

    in_maps = []
    for s in range(NCORES):
        sl = slice(s * CS, (s + 1) * CS)
        onehot = np.zeros((B, NSLOT), np.float32)
        if use_device_repl:
            rows = slot_rows[s]
            onehot[rows, np.arange(len(rows))] = 1.0
        in_maps.append({
            "q": np.ascontiguousarray(queue[:, sl]),
            "q2": np.ascontiguousarray(queue_2[:, sl]),
            "q3": np.ascontiguousarray(queue_3[:, sl]),
            "q4": np.ascontiguousarray(queue_4[:, sl]),
            "feat": f,
            "ema": ema,
            "prob": prob,
            "probe": prob_ema,
            "onehot": onehot,
        })

    res = run_bass_kernel_spmd(
        nc, in_maps, core_ids=list(range(NCORES)), trace=TRACE
    )
    global LAST_RESULT
    LAST_RESULT = res
    outs = res.results

    # ---- assemble bulk outputs ----
    q = np.empty((F, C), np.float32)
    q2 = np.empty((F, C), np.float32)
    q3 = np.empty((P, C), np.float32)
    q4 = np.empty((P, C), np.float32)
    for s in range(NCORES):
        sl = slice(s * CS, (s + 1) * CS)
        q[:, sl] = outs[s]["q_out"]
        q2[:, sl] = outs[s]["q2_out"]
        q3[:, sl] = outs[s]["q3_out"]
        q4[:, sl] = outs[s]["q4_out"]

    # ---- column updates (scatter the <=256 replaced columns) ----
    if use_device_repl:
        for s in range(NCORES):
            rows = slot_rows[s]
            if len(rows) == 0:
                continue
            cols = targets[rows]
            n = len(rows)
            q[:, cols] = outs[s]["repl1"][:, :n]
            q2[:, cols] = outs[s]["repl2"][:, :n]
            q3[:, cols] = outs[s]["repl3"][:, :n]
            q4[:, cols] = outs[s]["repl4"][:, :n]
    else:  # fallback: host-normalized replacements (pathological target skew)
        def hnorm(x):
            return x / np.sqrt((x * x).sum(axis=1, keepdims=True))
        q[:, targets] = hnorm(ema).T
        q2[:, targets] = hnorm(f).T
        q3[:, targets] = hnorm(prob).T
        q4[:, targets] = hnorm(prob_ema).T

    # ---- merge per-shard stats ----
    l_pos = outs[0]["lpos"][:, 0]  # [B]
    shard_vals = np.stack([outs[s]["cand_vals"] for s in range(NCORES)])  # [S,B,8]
    shard_idx = np.stack(
        [outs[s]["cand_idx"].astype(np.int64) for s in range(NCORES)]
    )  # [S,B,8] local col idx
    shard_se = np.stack([outs[s]["sumexp"][:, 0] for s in range(NCORES)])  # [S,B]
    m_s = shard_vals[:, :, 0]  # [S,B] per-shard max of l_neg

    # candidates in concat space (col 0 = l_pos, cols 1.. = l_neg).
    # l_pos candidate goes FIRST so argsort tie-break matches jax (lowest
    # index first on ties).
    cvals = np.concatenate(
        [l_pos[:, None]]
        + [shard_vals[s] for s in range(NCORES)], axis=1
    )  # [B, 65]
    cidx = np.concatenate(
        [np.zeros((B, 1), np.int64)]
        + [shard_idx[s] + s * CS + 1 for s in range(NCORES)], axis=1
    )  # [B, 65]
    order = np.argsort(-cvals, axis=1, kind="stable")[:, :KNN + 1]
    rows_ar = np.arange(B)[:, None]
    topv = cvals[rows_ar, order]  # [B, 7] descending
    topi = cidx[rows_ar, order]  # [B, 7]

    # global logsumexp of x = logits/BETA
    M = m_s.max(axis=0)  # [B]
    S_total = (shard_se * np.exp((m_s - M[None, :]) / BETA)).sum(axis=0)
    S_total = S_total + np.exp((l_pos - M) / BETA)
    lse = M / BETA + np.log(S_total)  # [B]

    # x[r, t_r]: t==0 -> l_pos else f_norm . queue[:, t-1]  (pre-update queue)
    fN_host = f / np.sqrt((f * f).sum(axis=1, keepdims=True))
    tcols = np.clip(targets - 1, 0, C - 1)
    qcols = queue[:, tcols]  # [F, B]
    tval = (fN_host * qcols.T).sum(axis=1)
    tval = np.where(targets == 0, l_pos, tval)
    xt = tval / BETA

    if KNN > 0 and epoch >= 20:
        w = _softmax_rows(topv[:, 1:] / BETA)  # [B, 6]
        neq = (topi[:, 1:] != targets[:, None])
        loss_rows = (lse - xt) + (w * (lse[:, None] - topv[:, 1:] / BETA) * neq).sum(
            axis=1
        )
        loss = np.float32(loss_rows.mean())
        oh = np.zeros((B, C + 1), np.float32)
        oh[rows_ar, topi[:, 1:]] = w.astype(np.float32)
        oh[np.arange(B), targets] = 1.0
        new_targets = oh
    else:
        loss = np.float32((lse - xt).mean())
        new_targets = targets_in.copy()

    return (loss, q, q2, q3, q4, new_targets)


# revision 17
# speedup vs baseline: 1.5675x; 1.0050x over previous
"""Trainium2 Bass kernel for nn_InvNet (topk_masking, memory-bound).

Contract: kernel(**inputs) takes FULL numpy inputs (as in setup_inputs()) and
returns the FULL output tuple (loss, q, q2, q3, q4, new_targets).

Sharding: class dimension C=32768 is split across 8 NeuronCores (4096 each).
Each core:
  - streams its class-shard of all four queues through SBUF (bulk copy out),
  - computes l_neg = f_norm @ queue_shard (fp32 PE matmul) reusing the same
    queue tiles,
  - reduces per-shard top-8 (values + indices), per-shard sum(exp(x - m)),
  - row-normalizes feature/prob matrices on device and emits the
    replacement-column blocks (normalized rows gathered by a one-hot matmul)
    used for the queue scatter update.
Host merges the tiny per-shard stats (8x8 top-k candidates, logsumexp terms),
computes the scalar loss, scatters the <=256 updated queue columns and the
~1.8k sparse one-hot weights into the outputs.
"""

import sys

sys.path.insert(0, "/opt/trn_rl_repo")

import numpy as np

import concourse.bass as bass
import concourse.tile as tile
from concourse import bacc, mybir
from concourse.masks import make_identity

B = 256
F = 2048
C = 32768
P = 1041
BETA = 0.05
KNN = 6
NCORES = 8
CS = C // NCORES  # 4096 classes per core
NSLOT = 64  # max updated columns handled on-device per core
F32 = mybir.dt.float32

_PROGRAM = None
TRACE = False  # set True (e.g. from test.py) to capture an NTFF profile
LAST_RESULT = None  # BassKernelResults of the most recent run


def _build_program():
    nc = bacc.Bacc("TRN2", target_bir_lowering=False, debug=False, num_devices=NCORES)

    q_in = nc.dram_tensor("q", [F, CS], F32, kind="ExternalInput")
    q2_in = nc.dram_tensor("q2", [F, CS], F32, kind="ExternalInput")
    q3_in = nc.dram_tensor("q3", [P, CS], F32, kind="ExternalInput")
    q4_in = nc.dram_tensor("q4", [P, CS], F32, kind="ExternalInput")
    f_in = nc.dram_tensor("feat", [B, F], F32, kind="ExternalInput")
    ema_in = nc.dram_tensor("ema", [B, F], F32, kind="ExternalInput")
    p_in = nc.dram_tensor("prob", [B, P], F32, kind="ExternalInput")
    pe_in = nc.dram_tensor("probe", [B, P], F32, kind="ExternalInput")
    oh_in = nc.dram_tensor("onehot", [B, NSLOT], F32, kind="ExternalInput")

    q_out = nc.dram_tensor("q_out", [F, CS], F32, kind="ExternalOutput")
    q2_out = nc.dram_tensor("q2_out", [F, CS], F32, kind="ExternalOutput")
    q3_out = nc.dram_tensor("q3_out", [P, CS], F32, kind="ExternalOutput")
    q4_out = nc.dram_tensor("q4_out", [P, CS], F32, kind="ExternalOutput")
    repl1 = nc.dram_tensor("repl1", [F, NSLOT], F32, kind="ExternalOutput")
    repl2 = nc.dram_tensor("repl2", [F, NSLOT], F32, kind="ExternalOutput")
    repl3 = nc.dram_tensor("repl3", [P, NSLOT], F32, kind="ExternalOutput")
    repl4 = nc.dram_tensor("repl4", [P, NSLOT], F32, kind="ExternalOutput")
    cand_vals = nc.dram_tensor("cand_vals", [B, 8], F32, kind="ExternalOutput")
    cand_idx = nc.dram_tensor("cand_idx", [B, 8], mybir.dt.uint32, kind="ExternalOutput")
    sumexp = nc.dram_tensor("sumexp", [B, 1], F32, kind="ExternalOutput")
    lpos = nc.dram_tensor("lpos", [B, 1], F32, kind="ExternalOutput")

    HALF = CS // 2  # 2048 col half-tiles for streaming

    with tile.TileContext(nc) as tc:
        from contextlib import ExitStack
        with ExitStack() as ctx:
            pp = ctx.enter_context(tc.tile_pool(name="persist", bufs=1))
            npool = ctx.enter_context(tc.tile_pool(name="normed", bufs=1))
            smalls = ctx.enter_context(tc.tile_pool(name="smalls", bufs=8))
            qp = ctx.enter_context(tc.tile_pool(name="qpool", bufs=10))
            cp = ctx.enter_context(tc.tile_pool(name="cpool", bufs=3))
            mps = ctx.enter_context(tc.tile_pool(name="mmpsum", bufs=8, space="PSUM"))

            acc = [pp.tile([128, CS], F32, tag=f"acc{m}", name=f"acc{m}")
                   for m in range(2)]
            fT = [pp.tile([128, B], F32, tag=f"fT{k}", name=f"fT{k}")
                  for k in range(F // 128)]
            ident = pp.tile([128, 128], F32, tag="ident", name="ident")
            make_identity(nc, ident[:])
            cv = pp.tile([128, 16], F32, tag="cv", name="cv")
            ci = pp.tile([128, 16], mybir.dt.uint32, tag="ci", name="ci")
            sv = pp.tile([128, 8], F32, tag="sv", name="sv")
            ohsb = pp.tile([128, 2 * NSLOT], F32, tag="ohsb", name="ohsb")

            def normalize(x_dram, d, m):
                """Load row-tile m of x and return a normalized SBUF tile."""
                r0, r1 = m * 128, min((m + 1) * 128, B)
                pr = r1 - r0
                raw = cp.tile([128, HALF], F32, tag="ctile", name="rawload")
                nc.sync.dma_start(out=raw[:pr, :d], in_=x_dram[r0:r1, :])
                sq = cp.tile([128, HALF], F32, tag="ctile", name="sqscratch")
                nv = smalls.tile([128, 4], F32, tag="nrm", name="nrm")
                nc.scalar.activation(
                    out=sq[:pr, :d], in_=raw[:pr, :d],
                    func=mybir.ActivationFunctionType.Square,
                    accum_out=nv[:pr, 0:1],
                )
                nc.scalar.activation(
                    out=nv[:pr, 1:2], in_=nv[:pr, 0:1],
                    func=mybir.ActivationFunctionType.Sqrt,
                )
                nc.vector.reciprocal(nv[:pr, 2:3], nv[:pr, 1:2])
                nt = npool.tile(
                    [128, d], F32, tag=f"normed_{x_dram.name}_{m}",
                    name=f"normed_{x_dram.name}_{m}",
                )
                nc.vector.tensor_scalar_mul(nt[:pr, :], raw[:pr, :d], nv[:pr, 2:3])
                return nt

            # --- critical path first: f -> fN -> fT (feeds all matmuls) ---
            fN = [normalize(f_in, F, m) for m in range(2)]
            for k in range(F // 128):
                for m in range(2):
                    pt = mps.tile([128, 512], F32, tag="mmps", name="tpsum")
                    nc.tensor.transpose(
                        out=pt[:, :128],
                        in_=fN[m][:, k * 128:(k + 1) * 128],
                        identity=ident[:],
                    )
                    nc.vector.tensor_copy(
                        out=fT[k][:, m * 128:(m + 1) * 128], in_=pt[:, :128]
                    )
            nc.sync.dma_start(out=ohsb[:, :NSLOT], in_=oh_in[0:128, :])
            nc.sync.dma_start(out=ohsb[:, NSLOT:], in_=oh_in[128:256, :])

            emaN = pN = peN = None

            def q_chunk(c):
                """Stream 2 row-tiles of `queue`: DMA in/out + matmul chunk.

                PSUM only fits 8 [128,512] banks, so accumulate k-pairs per
                m-tile sequentially: m0 matmuls, evict to acc, then m1.
                """
                tiles = []
                for kk in range(2):
                    k = 2 * c + kk
                    for h in range(2):
                        qt = qp.tile([128, HALF], F32, tag="qtile", name="qtile")
                        nc.sync.dma_start(
                            out=qt[:],
                            in_=q_in[k * 128:(k + 1) * 128,
                                     h * HALF:(h + 1) * HALF],
                        )
                        nc.gpsimd.dma_start(
                            out=q_out[k * 128:(k + 1) * 128,
                                      h * HALF:(h + 1) * HALF],
                            in_=qt[:],
                        )
                        tiles.append(qt)
                for m in range(2):
                    psums = [
                        mps.tile([128, 512], F32, tag="mmps", name="mmps")
                        for _ in range(8)
                    ]
                    for kk in range(2):
                        k = 2 * c + kk
                        for nb in range(8):
                            nc.tensor.matmul(
                                out=psums[nb][:],
                                lhsT=fT[k][:, m * 128:(m + 1) * 128],
                                rhs=tiles[2 * kk + nb // 4][
                                    :, (nb % 4) * 512:(nb % 4 + 1) * 512
                                ],
                                start=(kk == 0),
                                stop=(kk == 1),
                            )
                    for nb in range(8):
                        dstap = acc[m][:, nb * 512:(nb + 1) * 512]
                        if c == 0:
                            nc.vector.tensor_copy(out=dstap, in_=psums[nb][:])
                        else:
                            nc.vector.tensor_tensor(
                                out=dstap, in0=dstap, in1=psums[nb][:],
                                op=mybir.AluOpType.add,
                            )

            def copy_unit(src, dst, t, h, rows):
                r0 = t * 128
                pr = min(128, rows - r0)
                nc.sync.dma_start(
                    out=dst[r0:r0 + pr, h * HALF:(h + 1) * HALF],
                    in_=src[r0:r0 + pr, h * HALF:(h + 1) * HALF],
                )

            def repl_unit(src_tiles, dst, d):
                nt = (d + 127) // 128
                for t in range(nt):
                    f0 = t * 128
                    mt = min(128, d - f0)
                    ps = mps.tile([128, 512], F32, tag="mmps", name="replpsum")
                    for m in range(2):
                        nc.tensor.matmul(
                            out=ps[:mt, :NSLOT],
                            lhsT=src_tiles[m][:, f0:f0 + mt],
                            rhs=ohsb[:, m * NSLOT:(m + 1) * NSLOT],
                            start=(m == 0),
                            stop=(m == 1),
                        )
                    rs = smalls.tile([128, NSLOT], F32, tag="replsb", name="replsb")
                    nc.vector.tensor_copy(out=rs[:mt, :], in_=ps[:mt, :NSLOT])
                    nc.gpsimd.dma_start(out=dst[f0:f0 + mt, :], in_=rs[:mt, :])

            # --- interleaved emission: q-chunks + most d2d copies mixed,
            # holding back a tail reserve of copies to cover the stats ---
            import itertools
            q2u = [(q2_in, q2_out, t, h, F) for t in range(16) for h in range(2)]
            q3u = [(q3_in, q3_out, t, h, P) for t in range(9) for h in range(2)]
            q4u = [(q4_in, q4_out, t, h, P) for t in range(9) for h in range(2)]
            mixed = [u for tri in itertools.zip_longest(q2u, q3u, q4u)
                     for u in tri if u is not None]
            reserve = mixed[-12:]
            mixed = mixed[:-12]
            per_step = (len(mixed) + 7) // 8
            for i in range(8):
                q_chunk(i)
                for u in mixed[i * per_step:(i + 1) * per_step]:
                    copy_unit(*u)
                if i == 1:
                    emaN = [normalize(ema_in, F, m) for m in range(2)]
                if i == 2:
                    pN = [normalize(p_in, P, m) for m in range(2)]
                    peN = [normalize(pe_in, P, m) for m in range(2)]
                if i == 3:
                    # l_pos = rowsum(fN * emaN)
                    for m in range(2):
                        prod = cp.tile([128, HALF], F32, tag="ctile", name="lposprod")
                        nc.vector.tensor_tensor(
                            out=prod[:], in0=fN[m][:], in1=emaN[m][:],
                            op=mybir.AluOpType.mult,
                        )
                        lp = smalls.tile([128, 4], F32, tag="nrm", name="lpossb")
                        nc.vector.tensor_reduce(
                            out=lp[:, 0:1], in_=prod[:], axis=mybir.AxisListType.X,
                            op=mybir.AluOpType.add,
                        )
                        nc.gpsimd.dma_start(
                            out=lpos[m * 128:(m + 1) * 128, :], in_=lp[:, 0:1]
                        )
                if i == 4:
                    repl_unit(emaN, repl1, F)
                    repl_unit(fN, repl2, F)
                if i == 5:
                    repl_unit(pN, repl3, P)
                    repl_unit(peN, repl4, P)

            # --- per-shard stats from acc ---
            for m in range(2):
                cvm = cv[:, m * 8:(m + 1) * 8]
                nc.vector.max(out=cvm, in_=acc[m][:])
                cim = ci[:, m * 8:(m + 1) * 8]
                nc.vector.max_index(out=cim, in_max=cvm, in_values=acc[m][:])
                bias = sv[:, m:m + 1]
                nc.vector.tensor_scalar_mul(bias, cv[:, m * 8:m * 8 + 1], -1.0 / BETA)
                se0 = sv[:, 2 + 2 * m:3 + 2 * m]
                se1 = sv[:, 3 + 2 * m:4 + 2 * m]
                for h in range(2):
                    esc = cp.tile([128, HALF], F32, tag="ctile", name="expscratch")
                    nc.scalar.activation(
                        out=esc[:],
                        in_=acc[m][:, h * HALF:(h + 1) * HALF],
                        func=mybir.ActivationFunctionType.Exp,
                        bias=bias,
                        scale=1.0 / BETA,
                        accum_out=(se0 if h == 0 else se1),
                    )
                sem = sv[:, 6 + m:7 + m]
                nc.vector.tensor_tensor(
                    out=sem, in0=se0, in1=se1, op=mybir.AluOpType.add
                )
                sl = slice(m * 128, (m + 1) * 128)
                nc.gpsimd.dma_start(out=cand_vals[sl, :], in_=cvm)
                nc.gpsimd.dma_start(out=cand_idx[sl, :], in_=cim)
                nc.gpsimd.dma_start(out=sumexp[sl, :], in_=sem)

            # tail reserve: keeps DMA busy while stats run
            for u in reserve:
                copy_unit(*u)

    nc.compile()
    return nc


def _get_program():
    global _PROGRAM
    if _PROGRAM is None:
        _PROGRAM = _build_program()
    return _PROGRAM


def _softmax_rows(x):
    m = x.max(axis=1, keepdims=True)
    e = np.exp(x - m)
    return e / e.sum(axis=1, keepdims=True)


def _install_ntff_hook():
    """This image's antenv lacks axon_hooks; shim it so trace=True works."""
    import types

    try:
        from antenv.axon_hooks import get_axon_ntff_profile_hook  # noqa: F401
        return
    except ImportError:
        pass
    if "/root/.axon_site" not in sys.path:
        sys.path.insert(0, "/root/.axon_site")
    try:
        from trn_agent_boot.trn_boot import _ntff_profile_via_ctypes

        hook = _ntff_profile_via_ctypes("/opt/axon/libaxon_pjrt.so")
    except Exception:
        hook = None
    import antenv

    mod = types.ModuleType("antenv.axon_hooks")
    mod.get_axon_ntff_profile_hook = lambda: hook
    mod.set_axon_ntff_profile_hook = lambda h: None
    sys.modules["antenv.axon_hooks"] = mod
    antenv.axon_hooks = mod


def kernel(**inputs):
    from concourse.bass_utils import run_bass_kernel_spmd

    if TRACE:
        _install_ntff_hook()

    f = np.ascontiguousarray(np.asarray(inputs["inputs_feature"], dtype=np.float32))
    ema = np.ascontiguousarray(np.asarray(inputs["inputs_ema"], dtype=np.float32))
    prob = np.ascontiguousarray(np.asarray(inputs["prob"], dtype=np.float32))
    prob_ema = np.ascontiguousarray(np.asarray(inputs["prob_ema"], dtype=np.float32))
    queue = np.asarray(inputs["queue"], dtype=np.float32)
    queue_2 = np.asarray(inputs["queue_2"], dtype=np.float32)
    queue_3 = np.asarray(inputs["queue_3"], dtype=np.float32)
    queue_4 = np.asarray(inputs["queue_4"], dtype=np.float32)
    targets_in = np.asarray(inputs["targets"])
    targets = targets_in.astype(np.int64)
    epoch = int(np.asarray(inputs["epoch"]))

    nc = _get_program()

    # host-side slot assignment: which rows' target columns live on which core
    owner = targets // CS
    slot_rows = [np.nonzero(owner == s)[0] for s in range(NCORES)]
    use_device_repl = all(len(r) <= NSLOT for r in slot_rows)

    in_maps = []
    for s in range(NCORES):
        sl = slice(s * CS, (s + 1) * CS)
        onehot = np.zeros((B, NSLOT), np.float32)
        if use_device_repl:
            rows = slot_rows[s]
            onehot[rows, np.arange(len(rows))] = 1.0
        in_maps.append({
            "q": np.ascontiguousarray(queue[:, sl]),
            "q2": np.ascontiguousarray(queue_2[:, sl]),
            "q3": np.ascontiguousarray(queue_3[:, sl]),
            "q4": np.ascontiguousarray(queue_4[:, sl]),
            "feat": f,
            "ema": ema,
            "prob": prob,
            "probe": prob_ema,
            "onehot": onehot,
        })

    res = run_bass_kernel_spmd(
        nc, in_maps, core_ids=list(range(NCORES)), trace=TRACE
    )
    global LAST_RESULT
    LAST_RESULT = res
    outs = res.results

    # ---- assemble bulk outputs ----
    q = np.empty((F, C), np.float32)
    q2 = np.empty((F, C), np.float32)
    q3 = np.empty((P, C), np.float32)
    q4 = np.empty((P, C), np.float32)
    for s in range(NCORES):
        sl = slice(s * CS, (s + 1) * CS)
        q[:, sl] = outs[s]["q_out"]
        q2[:, sl] = outs[s]["q2_out"]
        q3[:, sl] = outs[s]["q3_out"]
        q4[:, sl] = outs[s]["q4_out"]

    # ---- column updates (scatter the <=256 replaced columns) ----
    if use_device_repl:
        for s in range(NCORES):
            rows = slot_rows[s]
            if len(rows) == 0:
                continue
            cols = targets[rows]
            n = len(rows)
            q[:, cols] = outs[s]["repl1"][:, :n]
            q2[:, cols] = outs[s]["repl2"][:, :n]
            q3[:, cols] = outs[s]["repl3"][:, :n]
            q4[:, cols] = outs[s]["repl4"][:, :n]
    else:  # fallback: host-normalized replacements (pathological target skew)
        def hnorm(x):
            return x / np.sqrt((x * x).sum(axis=1, keepdims=True))
        q[:, targets] = hnorm(ema).T
        q2[:, targets] = hnorm(f).T
        q3[:, targets] = hnorm(prob).T
        q4[:, targets] = hnorm(prob_ema).T

    # ---- merge per-shard stats ----
    l_pos = outs[0]["lpos"][:, 0]  # [B]
    shard_vals = np.stack([outs[s]["cand_vals"] for s in range(NCORES)])  # [S,B,8]
    shard_idx = np.stack(
        [outs[s]["cand_idx"].astype(np.int64) for s in range(NCORES)]
    )  # [S,B,8] local col idx
    shard_se = np.stack([outs[s]["sumexp"][:, 0] for s in range(NCORES)])  # [S,B]
    m_s = shard_vals[:, :, 0]  # [S,B] per-shard max of l_neg

    # candidates in concat space (col 0 = l_pos, cols 1.. = l_neg).
    # l_pos candidate goes FIRST so argsort tie-break matches jax (lowest
    # index first on ties).
    cvals = np.concatenate(
        [l_pos[:, None]]
        + [shard_vals[s] for s in range(NCORES)], axis=1
    )  # [B, 65]
    cidx = np.concatenate(
        [np.zeros((B, 1), np.int64)]
        + [shard_idx[s] + s * CS + 1 for s in range(NCORES)], axis=1
    )  # [B, 65]
    order = np.argsort(-cvals, axis=1, kind="stable")[:, :KNN + 1]
    rows_ar = np.arange(B)[:, None]
    topv = cvals[rows_ar, order]  # [B, 7] descending
    topi = cidx[rows_ar, order]  # [B, 7]

    # global logsumexp of x = logits/BETA
    M = m_s.max(axis=0)  # [B]
    S_total = (shard_se * np.exp((m_s - M[None, :]) / BETA)).sum(axis=0)
    S_total = S_total + np.exp((l_pos - M) / BETA)
    lse = M / BETA + np.log(S_total)  # [B]

    # x[r, t_r]: t==0 -> l_pos else f_norm . queue[:, t-1]  (pre-update queue)
    fN_host = f / np.sqrt((f * f).sum(axis=1, keepdims=True))
    tcols = np.clip(targets - 1, 0, C - 1)
    qcols = queue[:, tcols]  # [F, B]
    tval = (fN_host * qcols.T).sum(axis=1)
    tval = np.where(targets == 0, l_pos, tval)
    xt = tval / BETA

    if KNN > 0 and epoch >= 20:
        w = _softmax_rows(topv[:, 1:] / BETA)  # [B, 6]
        neq = (topi[:, 1:] != targets[:, None])
        loss_rows = (lse - xt) + (w * (lse[:, None] - topv[:, 1:] / BETA) * neq).sum(
            axis=1
        )
        loss = np.float32(loss_rows.mean())
        oh = np.zeros((B, C + 1), np.float32)
        oh[rows_ar, topi[:, 1:]] = w.astype(np.float32)
        oh[np.arange(B), targets] = 1.0
        new_targets = oh
    else:
        loss = np.float32((lse - xt).mean())
        new_targets = targets_in.copy()

    return (loss, q, q2, q3, q4, new_targets)
